# revision 22
# baseline (speedup 1.0000x reference)
"""GRU cell on 8 Trainium2 NeuronCores — data-parallel over batch, fp8 matmuls.

Math (per batch row):
    z = sigmoid([x, h] @ W_z + b_z)
    r = sigmoid([x, h] @ W_r + b_r)
    n = tanh(x @ W_n[:D] + (r * h) @ W_n[D:] + b_n)
    h' = (1 - z) * h + z * n = h + z * (n - h)

Distribution: batch 8192 is split 1024 rows per core; weights are
replicated. Everything on-device is computed in a transposed layout
[hidden, batch] so both matmul operands have the contraction dim on
SBUF partitions and no on-device transpose is needed:
    out.T[ho, b] = sum_k W[k, ho] * xh.T[k, b]
The host pre-transposes x/h (free) and transposes the result back.

Matmuls run in fp8_e4m3 with DoubleRow perf mode (2 contraction
chunks of 128 per instruction, 0.5 cycles/row = 157 TF/s, 2x bf16).
W entries are ~U(-1/64, 1/64), which sits at e4m3's min-normal, so
weights are pre-scaled by 64 on the host and the matmul result is
descaled by 1/64 in the activation (out = act(psum/64 + bias)).
The elementwise pipeline (gates, combine, h copy, output) runs in
bf16 (2x DVE throughput, half the DMA bytes); its quantization is
negligible next to the fp8 matmul error (~1.65e-2 vs the 2e-2 gate).
"""

import os
import sys
import types

import numpy as np

import concourse.bass as bass
import concourse.tile as tile
from concourse import bacc, mybir
from concourse._compat import with_exitstack
from concourse.bass_interp import get_hw_module
from concourse.bass_utils import run_bass_kernel_spmd

from ml_dtypes import bfloat16 as np_bf16
from ml_dtypes import float8_e4m3 as np_fp8

N_CORES = 8
D = 2048  # input size
H = 2048  # hidden size
BATCH = 8192
BC = BATCH // N_CORES  # batch per core (1024)
K = D + H  # contraction dim (4096)
P = 128  # partitions
KT = K // P  # k-chunks (32)
KK = KT // 2  # double-row k-pairs over [x, h] (16)
DT = D // P  # k-chunks covering the x part (16)
DK = DT // 2  # double-row k-pairs covering the x part (8)
JT = H // P  # hidden-out tiles (16)
NF = 512  # moving free dim per matmul (one PSUM bank of fp32)
NB = BC // NF  # batch blocks per core (2)
WSCALE = 64.0  # host-side weight pre-scale (descaled in activation)

f32 = mybir.dt.float32
bf16 = mybir.dt.bfloat16
fp8 = mybir.dt.float8e4
DR = mybir.MatmulPerfMode.DoubleRow


def _install_ntff_hook():
    """antenv.axon_hooks isn't injected in this image; shim it so
    run_bass_kernel_spmd(trace=True) can capture NTFF profiles."""
    if "antenv.axon_hooks" in sys.modules:
        return
    try:
        from trn_agent_boot.trn_boot import _ntff_profile_via_ctypes

        hook = _ntff_profile_via_ctypes("/opt/axon/libaxon_pjrt.so")
    except Exception:
        hook = None
    mod = types.ModuleType("antenv.axon_hooks")
    mod.get_axon_ntff_profile_hook = lambda: hook
    mod.set_axon_ntff_profile_hook = lambda h: None
    sys.modules["antenv.axon_hooks"] = mod


@with_exitstack
def _gru_tile_kernel(ctx, tc, xh, h32, wz, wr, wn, bz, br, bn, out):
    nc = tc.nc
    Sigmoid = mybir.ActivationFunctionType.Sigmoid
    Tanh = mybir.ActivationFunctionType.Tanh

    const_pool = ctx.enter_context(tc.tile_pool(name="const", bufs=1))
    xh_pool = ctx.enter_context(tc.tile_pool(name="xhp", bufs=1))
    h32_pool = ctx.enter_context(tc.tile_pool(name="h32p", bufs=1))
    rh_pool = ctx.enter_context(tc.tile_pool(name="rhp", bufs=1))
    w_pool = ctx.enter_context(tc.tile_pool(name="wp", bufs=12))
    act_pool = ctx.enter_context(tc.tile_pool(name="actp", bufs=2))
    out_pool = ctx.enter_context(tc.tile_pool(name="outp", bufs=3))
    psum_pool = ctx.enter_context(tc.tile_pool(name="psp", bufs=8, space="PSUM"))

    xh_sb = xh_pool.tile([P, KT, BC], fp8, name="xh_sb")
    h32_sb = h32_pool.tile([P, JT, BC], bf16, name="h32_sb")
    # r * h_prev (transposed) in fp8, filled during the r phase.
    rh_sb = rh_pool.tile([P, JT, BC], fp8, name="rh_sb")

    def load_w_cols(w_ap, j, name, nway=2, psplit=1):
        """[128, KT, 128] tile: [:, t, m] = W[t*128+p, j*128+m] * 64.
        Split so the transfer runs on several DMA engines; issued from
        the (otherwise idle) GpSimd queue so the Sync engine's serial
        ~600ns-per-descriptor stream stays free for xh/h/out."""
        wt = w_pool.tile([P, KT, P], fp8, tag="w", name=name)
        step = KT // nway
        pp = P // psplit
        for s in range(nway):
            for q in range(psplit):
                nc.gpsimd.dma_start(
                    wt[q * pp : (q + 1) * pp, s * step : (s + 1) * step, :],
                    w_ap[j, q * pp : (q + 1) * pp, s * step : (s + 1) * step, :],
                )
        return wt

    # DMA issue order is latency-critical: the engines drain the
    # gpsimd/sync descriptor queues roughly in issue order at HBM-bound
    # aggregate bandwidth, and each engine issues one descriptor per
    # ~600 ns. The first xh slice and the j=0 weight tile gate the
    # first matmul, so they lead their queues; the rest of xh follows
    # (j=0 is paced by its chunks); h32 only feeds the rh product /
    # combine whose real deadline is the NZ phase, so it trickles in
    # during the R loop.
    xh_flat = xh_sb[:].rearrange("p t n -> p (t n)")
    xw = KT * BC // 8
    nc.sync.dma_start(xh_flat[:, 0:xw], xh[:, 0:xw])
    wr_tiles = [load_w_cols(wr, 0, "wr_j", nway=2, psplit=2)]
    for s in range(1, 8):
        nc.sync.dma_start(xh_flat[:, s * xw : (s + 1) * xw], xh[:, s * xw : (s + 1) * xw])
    # Biases as [128, JT]: column j holds bias[j*128 : (j+1)*128].
    bias_sb = {}
    for name, ap in (("z", bz), ("r", br), ("n", bn)):
        t = const_pool.tile([P, JT], f32, name=f"bias_{name}")
        nc.sync.dma_start(t[:], ap.rearrange("(j p) -> p j", p=P))
        bias_sb[name] = t
    for jj in range(1, 5):
        wr_tiles.append(load_w_cols(wr, jj, "wr_j", nway=4))
    h32_flat = h32_sb[:].rearrange("p t n -> p (t n)")

    def load_h32_slice(s, nslices=8):
        w = JT * BC // nslices
        nc.sync.dma_start(h32_flat[:, s * w : (s + 1) * w], h32[:, s * w : (s + 1) * w])

    def accumulate(ps, w_tile, rhs_of_kk):
        """DoubleRow with full-width stationary [128, 2, 128]: the PE
        virtualizes to 256(k) x 128(out), 2 fp8 MACs/cell/cycle.
        16 k-pairs x 2 b_i accumulate into ps[b_i] [128, NF]."""
        for kk in range(KK):
            lhsT = w_tile[:, 2 * kk : 2 * kk + 2, :]
            for b_i in range(NB):
                nc.tensor.matmul(
                    ps[b_i][:],
                    lhsT,
                    rhs_of_kk(kk, b_i),
                    start=(kk == 0),
                    stop=(kk == KK - 1),
                    perf_mode=DR,
                )

    def xh_rhs(kk, b_i):
        return xh_sb[:, 2 * kk : 2 * kk + 2, b_i * NF : (b_i + 1) * NF]

    def n_rhs(kk, b_i):
        if kk < DK:
            return xh_rhs(kk, b_i)
        tt = kk - DK
        return rh_sb[:, 2 * tt : 2 * tt + 2, b_i * NF : (b_i + 1) * NF]

    def new_ps(name):
        return [psum_pool.tile([P, NF], f32, tag="ps", name=name) for _ in range(NB)]

    def act_gate(dst, ps, func, bias_col):
        """dst [128, BC] <- act(ps/WSCALE + bias)."""
        for b_i in range(NB):
            nc.scalar.activation(
                dst[:, b_i * NF : (b_i + 1) * NF],
                ps[b_i][:],
                func,
                bias=bias_col,
                scale=1.0 / WSCALE,
            )

    # ---- phase R: r gate, then rh = r * h_prev ----
    wz_tiles, wn_tiles = [], []
    for j in range(JT):
        wr_j = wr_tiles[j]
        if j + 5 < JT:
            wr_tiles.append(load_w_cols(wr, j + 5, "wr_j", nway=4))
        elif j >= JT - 4:
            idx = j - (JT - 4)
            wz_tiles.append(load_w_cols(wz, idx, "wz_j", nway=4))
            wn_tiles.append(load_w_cols(wn, idx, "wn_j", nway=4))
        if j % 2 == 0 and j // 2 < 8:
            load_h32_slice(j // 2)
        ps = new_ps("ps_r")
        accumulate(ps, wr_j, xh_rhs)
        r_j = act_pool.tile([P, BC], bf16, tag="r", name="r_j")
        act_gate(r_j, ps, Sigmoid, bias_sb["r"][:, j : j + 1])
        nc.vector.tensor_mul(rh_sb[:, j, :], r_j[:], h32_sb[:, j, :])

    # ---- phase NZ: z and n gates + combine ----
    for j in range(JT):
        wz_j, wn_j = wz_tiles[j], wn_tiles[j]
        if j + 4 < JT:
            wz_tiles.append(load_w_cols(wz, j + 4, "wz_j", nway=4))
            wn_tiles.append(load_w_cols(wn, j + 4, "wn_j", nway=4))
        # z fully accumulates + activates before n's psum tiles are
        # claimed, so 8 banks still give j-to-j+1 double buffering.
        z_j = act_pool.tile([P, BC], bf16, tag="z", name="z_j")
        n_j = act_pool.tile([P, BC], bf16, tag="n", name="n_j")
        psz = new_ps("ps_z")
        accumulate(psz, wz_j, xh_rhs)
        act_gate(z_j, psz, Sigmoid, bias_sb["z"][:, j : j + 1])
        psn = new_ps("ps_n")
        accumulate(psn, wn_j, n_rhs)
        act_gate(n_j, psn, Tanh, bias_sb["n"][:, j : j + 1])

        # h' = h + z * (n - h), per batch half for finer overlap of the
        # combine + output DMA with the next j's matmuls. The last j's
        # output DMAs split finer: they are the kernel's tail.
        for b_i in range(NB):
            sl = slice(b_i * NF, (b_i + 1) * NF)
            d_j = act_pool.tile([P, NF], bf16, tag="d", name="d_j")
            nc.vector.tensor_sub(d_j[:], n_j[:, sl], h32_sb[:, j, sl])
            zd_j = act_pool.tile([P, NF], bf16, tag="zd", name="zd_j")
            nc.vector.tensor_mul(zd_j[:], z_j[:, sl], d_j[:])
            o_j = out_pool.tile([P, NF], bf16, name="o_j")
            nc.vector.tensor_add(o_j[:], zd_j[:], h32_sb[:, j, sl])
            nway = 4 if j == JT - 1 and b_i == NB - 1 else (2 if j == JT - 1 else 1)
            step = NF // nway
            for s in range(nway):
                csl = slice(b_i * NF + s * step, b_i * NF + (s + 1) * step)
                nc.sync.dma_start(
                    out[j * P : (j + 1) * P, csl], o_j[:, s * step : (s + 1) * step]
                )


_CACHED = None


def _build():
    global _CACHED
    if _CACHED is not None:
        return _CACHED
    nc = bacc.Bacc(
        "TRN2", target_bir_lowering=False, debug=False, enable_asserts=False
    )
    # xh/h32 arrive pre-packed partition-major ([P, chunks*BC]) so the
    # per-partition DMA lines are long and contiguous.
    xh = nc.dram_tensor("xh", [P, KT * BC], fp8, kind="ExternalInput").ap()
    h32 = nc.dram_tensor("h32", [P, JT * BC], bf16, kind="ExternalInput").ap()
    # Weights pre-arranged on host: [JT, P, KT, P] where
    # w[j, p, t, m] = W[t*128+p, j*128+m] * 64, so the per-j DMA is a
    # fully contiguous [128, 4096] block (4 KiB per partition line).
    wz = nc.dram_tensor("wz", [JT, P, KT, P], fp8, kind="ExternalInput").ap()
    wr = nc.dram_tensor("wr", [JT, P, KT, P], fp8, kind="ExternalInput").ap()
    wn = nc.dram_tensor("wn", [JT, P, KT, P], fp8, kind="ExternalInput").ap()
    bz = nc.dram_tensor("bz", [H], f32, kind="ExternalInput").ap()
    br = nc.dram_tensor("br", [H], f32, kind="ExternalInput").ap()
    bn = nc.dram_tensor("bn", [H], f32, kind="ExternalInput").ap()
    out = nc.dram_tensor("out", [H, BC], bf16, kind="ExternalOutput").ap()

    with tile.TileContext(nc) as tc:
        _gru_tile_kernel(tc, xh, h32, wz, wr, wn, bz, br, bn, out)
    nc.compile()
    nc.m = get_hw_module(nc.m)
    _CACHED = nc
    return nc


def _pack_weight(W):
    """[K, H] f32 -> [JT, P, KT, P] fp8 with w[j,p,t,m] = W[t*128+p, j*128+m]*64."""
    w8 = (np.asarray(W, np.float32) * WSCALE).astype(np_fp8)
    w8 = w8.reshape(KT, P, JT, P).transpose(2, 1, 0, 3)
    return np.ascontiguousarray(w8)


def _make_in_maps(x, h_prev, W_z, b_z, W_r, b_r, W_n, b_n):
    wz8 = _pack_weight(W_z)
    wr8 = _pack_weight(W_r)
    wn8 = _pack_weight(W_n)
    bz32 = np.ascontiguousarray(np.asarray(b_z, np.float32))
    br32 = np.ascontiguousarray(np.asarray(b_r, np.float32))
    bn32 = np.ascontiguousarray(np.asarray(b_n, np.float32))
    in_maps = []
    for i in range(N_CORES):
        sl = slice(i * BC, (i + 1) * BC)
        xt = np.asarray(x[sl], np.float32).T
        ht = np.asarray(h_prev[sl], np.float32).T
        xh_i = np.concatenate([xt, ht], axis=0).astype(np_fp8)
        # pack [K, BC] -> [P, KT*BC] and [H, BC] -> [P, JT*BC]
        xh_p = xh_i.reshape(KT, P, BC).transpose(1, 0, 2).reshape(P, KT * BC)
        h16 = ht.astype(np_bf16)
        h32_p = h16.reshape(JT, P, BC).transpose(1, 0, 2).reshape(P, JT * BC)
        in_maps.append(
            {
                "xh": np.ascontiguousarray(xh_p),
                "h32": np.ascontiguousarray(h32_p),
                "wz": wz8,
                "wr": wr8,
                "wn": wn8,
                "bz": bz32,
                "br": br32,
                "bn": bn32,
            }
        )
    return in_maps


LAST_RESULT = None


def kernel(x, h_prev, W_z, b_z, W_r, b_r, W_n, b_n):
    global LAST_RESULT
    trace = bool(os.environ.get("GRU_TRACE"))
    if trace:
        _install_ntff_hook()
    nc = _build()
    in_maps = _make_in_maps(x, h_prev, W_z, b_z, W_r, b_r, W_n, b_n)
    res = run_bass_kernel_spmd(
        nc, in_maps, core_ids=list(range(N_CORES)), trace=trace
    )
    LAST_RESULT = res
    outs = [res.results[i]["out"].T for i in range(N_CORES)]
    return np.ascontiguousarray(np.concatenate(outs, axis=0).astype(np.float32))
